# revision 3
# baseline (speedup 1.0000x reference)
"""Trainium2 Bass kernel for nn_Condensation: 10 sequential masked-Gaussian-blur
composites over a [16,3,768,768] image, data-parallel over 8 NeuronCores.

Strategy (per core, 2 images = 6 image-channels):
  - out state kept resident in SBUF as bf16 [128, 6, 768] per image-channel.
  - Per drop, work restricted to the mask's support box (mask < ~1e-5 outside).
  - Separable blur done as two banded-matmul passes on TensorE (bf16, f32 PSUM):
      pass A: vT[w, h'] = sum_h om[h, w] * M^T[h, h']   (image block stationary)
      pass B: B[h', w'] = sum_w vT[w, h'] * M^T[w, w']  (vT block stationary)
    so no explicit transposes are needed and orientation is preserved.
  - Masks and conv matrices are computed exactly on host (numpy) from the
    runtime positions/radius inputs and DMA'd in per drop.
  - Composite out += m * (B - out) on VectorE in bf16 (2x mode).
"""
import numpy as np
import ml_dtypes

NUM_DROPS = 10
MIN_R, MAX_R = 60.0, 80.0
BETA = 1.8
BLUR_RADII = [11.3535, 17.9381, 5.7966, 10.8586, 5.5301, 15.9075, 12.3225, 13.4871, 6.6639, 9.5413]


def _ksize(r):
    k = int(2 * r) + 1
    return k + 1 if k % 2 == 0 else k


KSIZES = [_ksize(r) for r in BLUR_RADII]
H = W = 768
B_TOTAL, C = 16, 3
N_CORES = 8
B_LOC = B_TOTAL // N_CORES          # 2 images per core
IC = B_LOC * C                      # 6 image-channels per core
P = 128
NBLK = H // P                       # 6 h-blocks per image
EPS = 1e-5                          # mask support threshold

_bf16 = ml_dtypes.bfloat16


def _conv_matrix(sigma, ksize, n=768):
    """n x n matrix Kmat with blur_1d(x) = Kmat @ x, matching the reference
    (correlation with normalized gaussian, 'reflect' padding)."""
    half = (ksize - 1) * 0.5
    xs = np.linspace(-half, half, ksize)
    pdf = np.exp(-0.5 * (xs / np.float64(sigma)) ** 2)
    k1 = (pdf / pdf.sum()).astype(np.float32).astype(np.float64)
    pad = ksize // 2
    Kmat = np.zeros((n, n), dtype=np.float64)
    idx = np.arange(n)[:, None] + np.arange(ksize)[None, :] - pad   # [n, ksize]
    idx = np.abs(idx)
    idx = np.where(idx >= n, 2 * n - 2 - idx, idx)
    np.add.at(Kmat, (np.repeat(np.arange(n), ksize), idx.ravel()),
              np.tile(k1, n))
    return Kmat.astype(np.float32)


class _Drop:
    pass


def _drop_meta(positions, radius):
    """Host-side per-drop geometry + tensors (shared across cores)."""
    pos = np.clip(np.asarray(positions, np.float32), -1.0, 1.0)
    rad = np.clip(np.asarray(radius, np.float32), MIN_R, MAX_R)
    hv = np.arange(H, dtype=np.float32)[:, None]
    wv = np.arange(W, dtype=np.float32)[None, :]
    drops = []
    for j in range(NUM_DROPS):
        x0 = (pos[j, 0] + 1.0) / 2.0 * W
        y0 = (pos[j, 1] + 1.0) / 2.0 * H
        wr = rad[j]
        hr = wr * np.float32(0.8)
        ks = KSIZES[j]
        p = ks // 2
        s = float(np.sqrt((-np.log(EPS)) ** (1.0 / BETA)))
        h0 = max(0, int(np.floor(y0 - s * hr)))
        h1 = min(H, int(np.ceil(y0 + s * hr)) + 1)
        w0 = max(0, int(np.floor(x0 - s * wr)))
        w1 = min(W, int(np.ceil(x0 + s * wr)) + 1)
        w0 &= ~1
        w1 = min(W, (w1 + 1) & ~1)
        HB0, HB1 = h0 // P, (h1 + P - 1) // P
        WB0 = max(0, w0 - p) // P
        WB1 = (min(W, w1 + p) + P - 1) // P
        d = _Drop()
        d.j, d.p = j, p
        d.h0, d.h1, d.w0, d.w1 = h0, h1, w0, w1
        d.HB0, d.HBn = HB0, HB1 - HB0
        d.WB0, d.WBn = WB0, WB1 - WB0
        d.HBs, d.HBw = HB0 * P, (HB1 - HB0) * P
        d.WBs, d.WBw = WB0 * P, (WB1 - WB0) * P
        d.Wr = w1 - w0

        # mask over padded aligned box, zero outside support
        dd = (hv[d.HBs:d.HBs + d.HBw] - y0) ** 2 / hr ** 2 + \
             (wv[:, d.WBs:d.WBs + d.WBw] - x0) ** 2 / wr ** 2
        m = np.clip(np.exp(-(dd.astype(np.float32) ** np.float32(BETA)) + np.float32(1e-10)), 0.0, 1.0)
        mz = np.zeros_like(m)
        mz[h0 - d.HBs:h1 - d.HBs, w0 - d.WBs:w1 - d.WBs] = \
            m[h0 - d.HBs:h1 - d.HBs, w0 - d.WBs:w1 - d.WBs]
        # SBUF layout [part, hb, w]
        d.m_np = np.ascontiguousarray(
            mz.reshape(d.HBn, P, d.WBw).transpose(1, 0, 2)).astype(_bf16)

        MT = _conv_matrix(BLUR_RADII[j], ks).T    # MT[src, dst]
        kv = MT[d.HBs:d.HBs + d.HBw, d.HBs:d.HBs + d.HBw]      # [h, h']
        d.kv_np = np.ascontiguousarray(
            kv.reshape(d.HBn, P, d.HBw).transpose(1, 0, 2)).astype(_bf16)
        kh = MT[d.WBs:d.WBs + d.WBw, w0:w1]                    # [w, w']
        d.kh_np = np.ascontiguousarray(
            kh.reshape(d.WBn, P, d.Wr).transpose(1, 0, 2)).astype(_bf16)
        drops.append(d)
    return drops


def _build_program(drops):
    from contextlib import ExitStack
    from concourse import bacc, tile, mybir

    f32 = mybir.dt.float32
    bf16 = mybir.dt.bfloat16

    nc = bacc.Bacc("TRN2", target_bir_lowering=False, debug=False,
                   num_devices=N_CORES)
    img_d = nc.declare_dram_parameter("img", [B_LOC, C, H, W], f32, False)
    out_d = nc.declare_dram_parameter("out", [B_LOC, C, H, W], f32, True)
    dparams = []
    for d in drops:
        m_d = nc.declare_dram_parameter(f"m{d.j}", [P, d.HBn, d.WBw], bf16, False)
        kv_d = nc.declare_dram_parameter(f"kv{d.j}", [P, d.HBn, d.HBw], bf16, False)
        kh_d = nc.declare_dram_parameter(f"kh{d.j}", [P, d.WBn, d.Wr], bf16, False)
        dparams.append((m_d, kv_d, kh_d))

    with tile.TileContext(nc) as tc, ExitStack() as ctx:
        outp = ctx.enter_context(tc.tile_pool(name="out_state", bufs=1))
        out_s = [outp.tile([P, NBLK, W], bf16, name=f"state{ic}", tag=f"state{ic}")
                 for ic in range(IC)]
        iop = ctx.enter_context(tc.tile_pool(name="io", bufs=4))
        dp = ctx.enter_context(tc.tile_pool(name="dropin", bufs=2))
        wp = ctx.enter_context(tc.tile_pool(name="work", bufs=3))
        pA = ctx.enter_context(tc.tile_pool(name="psA", bufs=3, space="PSUM"))
        pB = ctx.enter_context(tc.tile_pool(name="psB", bufs=3, space="PSUM"))

        # ---- load img (f32) -> out_s (bf16)
        for ic in range(IC):
            b, c = divmod(ic, C)
            for hb in range(NBLK):
                stage = iop.tile([P, W], f32, tag="stg_in")
                nc.sync.dma_start(out=stage[:],
                                  in_=img_d.ap()[b, c, hb * P:(hb + 1) * P, :])
                nc.any.tensor_copy(out_s[ic][:, hb, :], stage[:])

        # ---- drops
        for d, (m_d, kv_d, kh_d) in zip(drops, dparams):
            m_t = dp.tile([P, d.HBn, d.WBw], bf16, tag="m")
            kv_t = dp.tile([P, d.HBn, d.HBw], bf16, tag="kv")
            kh_t = dp.tile([P, d.WBn, d.Wr], bf16, tag="kh")
            nc.sync.dma_start(out=m_t[:], in_=m_d.ap()[:])
            nc.sync.dma_start(out=kv_t[:], in_=kv_d.ap()[:])
            nc.sync.dma_start(out=kh_t[:], in_=kh_d.ap()[:])
            for ic in range(IC):
                om = wp.tile([P, d.HBn, d.WBw], bf16, tag="om")
                nc.vector.tensor_mul(
                    om[:], m_t[:],
                    out_s[ic][:, d.HB0:d.HB0 + d.HBn, d.WBs:d.WBs + d.WBw])
                # pass A: vT[w-chunk, h'] over HB window
                vt = wp.tile([P, d.WBn, d.HBw], bf16, tag="vt")
                for wc in range(d.WBn):
                    ps = pA.tile([P, d.HBw], f32, tag="psA")
                    for k in range(d.HBn):
                        nc.tensor.matmul(
                            ps[:],
                            lhsT=om[:, k, wc * P:(wc + 1) * P],
                            rhs=kv_t[:, k, :],
                            start=(k == 0), stop=(k == d.HBn - 1))
                    nc.scalar.copy(vt[:, wc, :], ps[:])
                # pass B: B[h'-block, w' in R.w]
                Bs = wp.tile([P, d.HBn, d.Wr], bf16, tag="Bs")
                for hb in range(d.HBn):
                    ps2 = pB.tile([P, d.Wr], f32, tag="psB")
                    for wc in range(d.WBn):
                        nc.tensor.matmul(
                            ps2[:],
                            lhsT=vt[:, wc, hb * P:(hb + 1) * P],
                            rhs=kh_t[:, wc, :],
                            start=(wc == 0), stop=(wc == d.WBn - 1))
                    nc.scalar.copy(Bs[:, hb, :], ps2[:])
                # composite: out += m * (B - out) over [HB rows, w0:w1]
                osl = out_s[ic][:, d.HB0:d.HB0 + d.HBn, d.w0:d.w1]
                mr = m_t[:, :, d.w0 - d.WBs:d.w0 - d.WBs + d.Wr]
                t1 = wp.tile([P, d.HBn, d.Wr], bf16, tag="t1")
                nc.vector.tensor_sub(t1[:], Bs[:], osl)
                t2 = wp.tile([P, d.HBn, d.Wr], bf16, tag="t2")
                nc.vector.tensor_mul(t2[:], mr, t1[:])
                nc.vector.tensor_add(osl, osl, t2[:])

        # ---- store out_s (bf16) -> out (f32)
        for ic in range(IC):
            b, c = divmod(ic, C)
            for hb in range(NBLK):
                stage = iop.tile([P, W], f32, tag="stg_out")
                nc.any.tensor_copy(stage[:], out_s[ic][:, hb, :])
                nc.sync.dma_start(out=out_d.ap()[b, c, hb * P:(hb + 1) * P, :],
                                  in_=stage[:])

    nc.compile()
    return nc


_CACHE = {}


def _get_program(positions, radius):
    key = (np.asarray(positions, np.float32).tobytes(),
           np.asarray(radius, np.float32).tobytes())
    if key not in _CACHE:
        drops = _drop_meta(positions, radius)
        _CACHE[key] = (_build_program(drops), drops)
    return _CACHE[key]


def kernel(img, positions, radius, _want_trace=False, **_kw):
    from concourse.bass_utils import run_bass_kernel_spmd
    img = np.ascontiguousarray(np.asarray(img, np.float32))
    assert img.shape == (B_TOTAL, C, H, W)
    nc, drops = _get_program(positions, radius)

    shards = img.reshape(N_CORES, B_LOC, C, H, W)
    base = {}
    for d in drops:
        base[f"m{d.j}"] = d.m_np
        base[f"kv{d.j}"] = d.kv_np
        base[f"kh{d.j}"] = d.kh_np
    in_maps = [dict(base, img=shards[i]) for i in range(N_CORES)]
    res = run_bass_kernel_spmd(nc, in_maps, core_ids=list(range(N_CORES)),
                               trace=_want_trace)
    out = np.concatenate([res.results[i]["out"] for i in range(N_CORES)], axis=0)
    out = out.reshape(B_TOTAL, C, H, W).astype(np.float32)
    if _want_trace:
        return out, res
    return out
